# revision 31
# baseline (speedup 1.0000x reference)
"""Distributed Trainium2 kernel: Gemma-style attention block (B=2,T=2048,H=2048,
NH=16,NKV=4,HD=128), tensor-parallel over heads across 8 NeuronCores.

Per core c: q heads {2c, 2c+1}, kv head c//2.  Activations are kept
feature-major ("transposed", [d_part, t_free]) so every matmul contracts on the
partition dim.  Softmax is max-free (safe: rmsnorm bounds |scores| <= sqrt(HD)),
denominators and rmsnorm sum-of-squares are computed pre-broadcast via an
all-ones 128x128 stationary matmul.

Host<->device traffic is minimized (it dominates end-to-end time):
  - x is token-sharded: each core receives a distinct [H, 512] bf16 slice and
    the full xT is reassembled on device via AllGather.
  - Weights ship as int8 with per-input-feature scales.  The qkv scales are
    folded into x on the host; the o_proj scales are folded into the
    softmax reciprocal, so the int8 weights are used directly (as exact
    bf16 integers) in the matmuls with zero extra passes.
  - rope cos/sin tables and the causal masks are generated on device
    (iota -> frac -> Sin activation); only the tiny 128x128 rot-half
    matrices and per-partition scale columns are shipped.
  - o_proj partials are ReduceScatter-summed on device, so each core returns
    only its [512, H] token chunk.
"""

import os
import sys

sys.path.insert(0, "/opt/trn_rl_repo")

import numpy as np
import ml_dtypes

import concourse.bass as bass
import concourse.mybir as mybir
import concourse.tile as tile
from concourse.bass_utils import run_bass_kernel_spmd

BF16 = ml_dtypes.bfloat16

B, T, H = 2, 2048, 2048
NH, NKV, HD = 16, 4, 128
THETA = 10000.0
EPS = 1e-6
NCORES = 8
QH = NH // NCORES          # 2 q heads per core
BT = B * T                 # 4096
NBLK = T // 512            # 4 blocks of 512 per batch
SCALE = 1.0 / np.sqrt(HD)
TOKS = BT // NCORES        # 512 tokens per core shard
TWO_PI = 2.0 * np.pi

LAST_RESULTS = None        # stash for test harness profiling

# column offsets inside the packed constants SBUF tile [128, NCONST]
OFF_WQKV = 0                      # 16*512 (from int8 input, converted)
OFF_WO = OFF_WQKV + 16 * 512      # QH*2048 (from int8 input, converted)
OFF_CQ = OFF_WO + QH * H          # 2048 (device-generated)
OFF_CK = OFF_CQ + T               # 2048 (device-generated)
OFF_SIN = OFF_CK + T              # 2048 (device-generated)
OFF_RQ = OFF_SIN + T              # 128 (shipped, negated)
OFF_RK = OFF_RQ + HD              # 128 (shipped, negated)
OFF_MASK = OFF_RK + HD            # 4*512 (device-generated)
OFF_ONES = OFF_MASK + 4 * 512     # 128 (device-generated)
NCONST = OFF_ONES + 128
NWC = OFF_WO + QH * H             # int8 weight input width (12288)

# wsc (f32 [128, NSC]) column map (the qkv input-feature scales s1 are
# folded into x on the host before shipping)
SC_S2 = 0        # QH cols: s2[h*128+p] — o_proj d_in scales
SC_IF = QH       # invfreq[p % 64] / (2*pi)
SC_CWQ = QH + 1  # 1 + q_norm_w[p]
SC_CWK = QH + 2  # 1 + k_norm_w[p]
NSC = QH + 3


def _rot_tables(w_q, w_k):
    """R_w = rot_half matrix with the ±1 and the (1+w) source weight folded
    in.  Returns rotmT_q, rotmT_k in lhsT layout."""
    rotms = []
    for w in (w_q, w_k):
        wp = 1.0 + w.astype(np.float64)
        R = np.zeros((HD, HD))
        for m in range(64):
            R[m, m + 64] = -wp[m + 64]
        for m in range(64, HD):
            R[m, m - 64] = +wp[m - 64]
        rotms.append(np.ascontiguousarray(R.T).astype(BF16))  # lhsT
    return rotms[0], rotms[1]


def _legalize_waits(nc):
    """This container's walrus accepts only ONE sync wait per instruction
    (even shipped Tile kernels fail codegen). Split each multi-wait
    instruction into single-wait NOPs on the same engine followed by the
    original holding the last wait — per-engine program order makes this
    exactly equivalent."""
    nid = 0
    for fn in nc.m.functions:
        for blk in fn.blocks:
            out = []
            for inst in blk.instructions:
                si = getattr(inst, "sync_info", None)
                if si is not None and si.on_wait and len(si.on_wait) > 1:
                    waits = list(si.on_wait)
                    ups = list(si.on_update) if si.on_update else []
                    for w in waits[:-1]:
                        nop = mybir.InstNoOp(name=f"swx-{nid}", ins=[], outs=[])
                        nid += 1
                        nop.engine = inst.engine
                        nop.sync_info = mybir.SyncInfo(on_wait=[w], on_update=[])
                        out.append(nop)
                    inst.sync_info = mybir.SyncInfo(
                        on_wait=[waits[-1]], on_update=ups)
                out.append(inst)
            blk.instructions = out
    return nc


def _build_graph(perturb=0, repeat=1, cfg=None):
    cfg = {**dict(xtp=24, tmp=6, pacc=2, pmm=4, depth=3), **(cfg or {})}
    nc = bass.Bass(num_devices=NCORES)
    f32, bf16 = mybir.dt.float32, mybir.dt.bfloat16
    i8, i32 = mybir.dt.int8, mybir.dt.int32
    GRP = [list(range(NCORES))]
    AF = mybir.ActivationFunctionType

    ROTW = 2 * HD // NCORES      # 32-col slice of the [rq|rk] block per core
    xs = nc.dram_tensor("xs", [H, TOKS], bf16, kind="ExternalInput")
    wc = nc.dram_tensor("wc", [128, NWC], i8, kind="ExternalInput")
    wsc = nc.dram_tensor("wsc", [128, NSC], f32, kind="ExternalInput")
    rot = nc.dram_tensor("rot", [128, ROTW], bf16, kind="ExternalInput")
    out = nc.dram_tensor("out", [TOKS, H], bf16, kind="ExternalOutput")

    with tile.TileContext(nc) as tc:
        with (
            tc.tile_pool(name="dram", bufs=1, space="DRAM") as dram,
            tc.tile_pool(name="singles", bufs=1) as singles,
            tc.tile_pool(name="xtp", bufs=cfg["xtp"]) as xtp,
            tc.tile_pool(name="tmp", bufs=cfg["tmp"]) as tmp,
            tc.tile_pool(name="psum", bufs=cfg["pacc"], space="PSUM") as pacc,
            tc.tile_pool(name="psmm", bufs=cfg["pmm"], space="PSUM") as pmm,
        ):
            # per-batch halves: AG_B and RS_0 overlap with compute
            HB = TOKS // 2          # 256 tokens per core per batch
            xs_b = [dram.tile([H, HB], bf16, name=f"xs_b{i}", tag=f"xs_b{i}")
                    for i in range(B)]
            xg = [dram.tile([NCORES * H, HB], bf16, name=f"xg{i}", tag=f"xg{i}")
                  for i in range(B)]
            part = dram.tile([BT, H], bf16, name="part", tag="part")
            out_b = [dram.tile([HB, H], bf16, name=f"out_b{i}", tag=f"out_b{i}")
                     for i in range(B)]

            rot_b = dram.tile([128, ROTW], bf16, name="rot_b", tag="rot_b")
            rot_g = dram.tile([NCORES * 128, ROTW], bf16, name="rot_g",
                              tag="rot_g")

            consts_sb = singles.tile([128, NCONST], bf16)
            wsc_sb = singles.tile([128, NSC], f32, name="wsc_sb", tag="wsc_sb")
            nc.sync.dma_start(out=wsc_sb, in_=wsc[:, :])

            wqkv_sb = consts_sb[:, OFF_WQKV:OFF_WQKV + 16 * 512]
            wo_sb = consts_sb[:, OFF_WO:OFF_WO + QH * H]
            cq_sb = consts_sb[:, OFF_CQ:OFF_CQ + T]
            ck_sb = consts_sb[:, OFF_CK:OFF_CK + T]
            sin_sb = consts_sb[:, OFF_SIN:OFF_SIN + T]
            rq_sb = consts_sb[:, OFF_RQ:OFF_RQ + HD]
            rk_sb = consts_sb[:, OFF_RK:OFF_RK + HD]
            mask_sb = consts_sb[:, OFF_MASK:OFF_MASK + 4 * 512]
            ones_sb = consts_sb[:, OFF_ONES:OFF_ONES + 128]

            # ---- AllGather the (host-prescaled) x slices, batch 0 first so
            # AG_1 hides behind phase1(0) compute; the tiny rot AG slots in
            # between (needed by phase1(0) rope, after AG_0 either way) ----
            nc.sync.dma_start(out=rot_b, in_=rot[:, :])
            for i in range(B):
                nc.sync.dma_start(
                    out=xs_b[i], in_=xs[:, i * HB:(i + 1) * HB])
                nc.gpsimd.collective_compute(
                    "AllGather", mybir.AluOpType.bypass, replica_groups=GRP,
                    ins=[xs_b[i][:].opt()], outs=[xg[i][:].opt()])
                if i == 0:
                    nc.gpsimd.collective_compute(
                        "AllGather", mybir.AluOpType.bypass,
                        replica_groups=GRP,
                        ins=[rot_b[:].opt()], outs=[rot_g[:].opt()])
            for r in range(NCORES):
                nc.sync.dma_start(
                    out=consts_sb[:, OFF_RQ + r * ROTW:OFF_RQ + (r + 1) * ROTW],
                    in_=rot_g[r * 128:(r + 1) * 128, :])

            # ---- int8 -> bf16 weight decode (values are exact integers) ----
            with tc.tile_pool(name="wstg", bufs=2) as wstg:
                for j in range(0, NWC, 2048):
                    stg = wstg.tile([128, 2048], i8, tag="stg")
                    nc.sync.dma_start(out=stg, in_=wc[:, j:j + 2048])
                    with nc.allow_low_precision(reason="int8 exact in bf16"):
                        nc.vector.tensor_copy(
                            out=consts_sb[:, j:j + 2048], in_=stg)

            # ---- device-generated rope tables, causal mask, ones ----
            # frac-center u into [-1/2, 1/2] via two f32->i32->f32 roundtrips
            # (robust to either trunc or round-nearest conversion), then
            # table = Sin(2*pi*w) = sin(emb) exactly (periodicity).
            CH = T // 2
            tf = singles.tile([128, CH], f32, name="tf", tag="tf")
            vf = singles.tile([128, CH], f32, name="vf", tag="vf")
            gf = singles.tile([128, CH], f32, name="gf", tag="gf")
            ii = singles.tile([128, CH], i32, name="ii", tag="ii")
            cosr = singles.tile([128, T], bf16, name="cosr", tag="cosr")

            def frac_sin(dst, bias):
                nc.scalar.activation(out=vf, in_=tf, func=AF.Copy,
                                     scale=wsc_sb[:, SC_IF:SC_IF + 1],
                                     bias=bias)
                for _ in range(2):
                    nc.vector.tensor_copy(out=ii, in_=vf)
                    nc.vector.tensor_copy(out=gf, in_=ii)
                    nc.vector.tensor_sub(out=vf, in0=vf, in1=gf)
                with nc.allow_low_precision(reason="rope table bf16, as v1"):
                    nc.scalar.activation(out=dst, in_=vf, func=AF.Sin,
                                         scale=TWO_PI)

            for ch in range(2):
                sl = slice(ch * CH, (ch + 1) * CH)
                nc.gpsimd.iota(tf[:], pattern=[[1, CH]], base=ch * CH,
                               channel_multiplier=0,
                               allow_small_or_imprecise_dtypes=True)
                frac_sin(sin_sb[:, sl], 0.0)           # sin(emb)
                frac_sin(cosr[:, sl], 0.25)            # cos(emb)
            with nc.allow_low_precision(reason="rope table bf16, same as v1"):
                nc.vector.tensor_scalar_mul(
                    cq_sb, cosr, wsc_sb[:, SC_CWQ:SC_CWQ + 1])
                nc.vector.tensor_scalar_mul(
                    ck_sb, cosr, wsc_sb[:, SC_CWK:SC_CWK + 1])
            # causal masks (4 shifted 512-blocks) + all-ones column
            nc.vector.memset(mask_sb, 1.0)
            nc.vector.memset(ones_sb, 1.0)
            for r in range(4):
                nc.gpsimd.affine_select(
                    out=mask_sb[:, r * 512:(r + 1) * 512],
                    in_=mask_sb[:, r * 512:(r + 1) * 512],
                    pattern=[[1, 512]], base=-128 * r, channel_multiplier=-1,
                    compare_op=mybir.AluOpType.is_ge, fill=0.0)
            for _ in range(perturb):
                nc.sync.nop()

            # ---- per-batch activations (feature-major) ----
            qT = [singles.tile([128, QH * T], bf16, name=f"qT{b}", tag=f"qT{b}")
                  for b in range(B)]
            kT = [singles.tile([128, T], bf16, name=f"kT{b}", tag=f"kT{b}")
                  for b in range(B)]
            vn = [singles.tile([128, 16 * 128], bf16, name=f"vn{b}", tag=f"vn{b}")
                  for b in range(B)]
            attnT = [singles.tile([128, QH * T], bf16, name=f"attnT{b}", tag=f"attnT{b}")
                     for b in range(B)]

            def phase1(b):
                for blk in range(NBLK):
                    t0 = blk * 512
                    xts = []
                    for ht in range(16):
                        xt_t = xtp.tile([128, 512], bf16, tag="xt")
                        for q2 in range(2):      # 256-token AG chunks
                            r = 2 * blk + q2
                            nc.sync.dma_start(
                                out=xt_t[:, q2 * HB:(q2 + 1) * HB],
                                in_=xg[b][r * H + ht * 128:
                                          r * H + (ht + 1) * 128, :])
                        xts.append(xt_t)
                    # q0, q1, k projections (feature-major out)
                    for dt in range(3):
                        ps = pacc.tile([128, 512], f32, tag="acc")
                        for ht in range(16):
                            nc.tensor.matmul(
                                ps,
                                lhsT=wqkv_sb[:, ht * 512 + dt * 128:ht * 512 + (dt + 1) * 128],
                                rhs=xts[ht], start=(ht == 0), stop=(ht == 15))
                        traw = tmp.tile([128, 512], bf16, tag="traw")
                        with nc.allow_low_precision(reason="bf16 act copy"):
                            nc.vector.tensor_copy(out=traw, in_=ps)
                        sq = tmp.tile([128, 512], bf16, tag="sq")
                        nc.vector.tensor_mul(sq, traw, traw)
                        ssq = pmm.tile([128, 512], f32, tag="mm")
                        nc.tensor.matmul(ssq, lhsT=ones_sb, rhs=sq, start=True, stop=True)
                        std = tmp.tile([128, 512], f32, tag="std")
                        nc.scalar.activation(
                            out=std, in_=ssq,
                            func=mybir.ActivationFunctionType.Sqrt,
                            scale=1.0 / HD)
                        rstd = tmp.tile([128, 512], bf16, tag="rstd")
                        with nc.allow_low_precision(reason="rstd bf16 ok at 2e-2 tol"):
                            nc.vector.reciprocal(out=rstd, in_=std)
                        cos_t, rot_t = (cq_sb, rq_sb) if dt < 2 else (ck_sb, rk_sb)
                        t1 = tmp.tile([128, 512], bf16, tag="t1")
                        nc.vector.tensor_mul(t1, traw, cos_t[:, t0:t0 + 512])
                        rps = pmm.tile([128, 512], f32, tag="mm")
                        nc.tensor.matmul(rps, lhsT=rot_t, rhs=traw, start=True, stop=True)
                        t2 = tmp.tile([128, 512], bf16, tag="t2")
                        nc.vector.tensor_mul(t2, rps, sin_sb[:, t0:t0 + 512])
                        nc.vector.tensor_add(out=t1, in0=t1, in1=t2)
                        dest = (qT[b][:, dt * T + t0:dt * T + t0 + 512] if dt < 2
                                else kT[b][:, t0:t0 + 512])
                        nc.vector.tensor_mul(dest, t1, rstd)
                    # v projection, natural layout [t_part, d_free]
                    vps = pacc.tile([128, 512], f32, tag="acc")
                    for c4 in range(4):
                        for ht in range(16):
                            nc.tensor.matmul(
                                vps[:, c4 * 128:(c4 + 1) * 128],
                                lhsT=xts[ht][:, c4 * 128:(c4 + 1) * 128],
                                rhs=wqkv_sb[:, ht * 512 + 384:ht * 512 + 512],
                                start=(ht == 0), stop=(ht == 15))
                    with nc.allow_low_precision(reason="bf16 act copy"):
                        nc.vector.tensor_copy(
                            out=vn[b][:, blk * 512:(blk + 1) * 512], in_=vps)

            def attn_block(b, h, j):
                # Software-pipelined: S^T matmuls issued DEPTH tiles ahead so
                # the PE never stalls on the ACT exp of the current tile.
                DEPTH = cfg["depth"]
                ntk = 4 * j + 4
                aps = pacc.tile([128, 512], f32, tag="acc")
                dps = pacc.tile([128, 512], f32, tag="den")
                sps_l, pt_l = [], []

                def issue_st(i):
                    sps = pmm.tile([128, 512], f32, tag="mm", name="sps")
                    nc.tensor.matmul(
                        sps, lhsT=kT[b][:, i * 128:(i + 1) * 128],
                        rhs=qT[b][:, h * T + j * 512:h * T + (j + 1) * 512],
                        start=True, stop=True)
                    sps_l.append(sps)

                def issue_exp(i):
                    pt = tmp.tile([128, 512], bf16, tag="pt", name="pt")
                    nc.scalar.activation(
                        out=pt, in_=sps_l[i],
                        func=mybir.ActivationFunctionType.Exp, scale=SCALE)
                    if i >= 4 * j:
                        r = i - 4 * j
                        nc.vector.tensor_mul(
                            pt, pt, mask_sb[:, r * 512:(r + 1) * 512])
                    pt_l.append(pt)

                for i in range(min(DEPTH, ntk)):
                    issue_st(i)
                issue_exp(0)
                for i in range(ntk):
                    if i + DEPTH < ntk:
                        issue_st(i + DEPTH)
                    if i + 1 < ntk:
                        issue_exp(i + 1)
                    nc.tensor.matmul(dps, lhsT=ones_sb, rhs=pt_l[i],
                                     start=(i == 0), stop=(i == ntk - 1))
                    nc.tensor.matmul(aps, lhsT=vn[b][:, i * 128:(i + 1) * 128],
                                     rhs=pt_l[i], start=(i == 0),
                                     stop=(i == ntk - 1))
                recip = tmp.tile([128, 512], mybir.dt.float32, tag="rec")
                nc.vector.reciprocal(out=recip, in_=dps)
                # fold the per-d_in o_proj dequant scale into the reciprocal
                nc.vector.tensor_scalar_mul(
                    recip, recip, wsc_sb[:, SC_S2 + h:SC_S2 + h + 1])
                nc.vector.tensor_mul(
                    attnT[b][:, h * T + j * 512:h * T + (j + 1) * 512], aps, recip)

            def phase2(b):
                for h in range(QH):
                    for j in range(NBLK):
                        attn_block(b, h, j)

            def oproj_tile(b, m, j):
                ops = pmm.tile([128, 512], f32, tag="mm", name="ops")
                for hh in range(QH):
                    nc.tensor.matmul(
                        ops,
                        lhsT=attnT[b][:, hh * T + m * 128:hh * T + (m + 1) * 128],
                        rhs=wo_sb[:, hh * H + j * 512:hh * H + (j + 1) * 512],
                        start=(hh == 0), stop=(hh == QH - 1))
                osb = tmp.tile([128, 512], bf16, tag="osb", name="osb")
                with nc.allow_low_precision(reason="bf16 partials, RS-summed"):
                    nc.vector.tensor_copy(out=osb, in_=ops)
                nc.sync.dma_start(
                    out=part[b * T + m * 128:b * T + (m + 1) * 128,
                             j * 512:(j + 1) * 512],
                    in_=osb)

            def phase3(b):
                for m in range(16):
                    for j in range(NBLK):
                        oproj_tile(b, m, j)

            def rs_batch(i):
                # each core keeps its 256-token chunk of batch i's sum
                nc.gpsimd.collective_compute(
                    "ReduceScatter", mybir.AluOpType.add, replica_groups=GRP,
                    ins=[part[i * T:(i + 1) * T, :].opt()],
                    outs=[out_b[i][:].opt()])
                nc.sync.dma_start(
                    out=out[i * HB:(i + 1) * HB, :], in_=out_b[i])

            for _ in range(repeat):   # >1 only for benchmarking (idempotent)
                phase1(0)
                phase2(0)
                phase1(1)
                phase3(0)
                rs_batch(0)           # overlaps phase2(1)/phase3(1)
                phase2(1)
                phase3(1)
                rs_batch(1)
    return nc


_GRAPH = None


_FAST_EXEC = None


def _build_fast_exec(nc):
    """Vendored multi-core branch of bass2jax.run_bass_via_pjrt with two
    changes: the donated zero output buffers are created on-device (tiny
    jitted broadcast, sharded over the 8 cores) instead of being shipped
    from the host (~17MB less H2D per call), and the jitted callables are
    built once and reused so repeat calls skip retracing."""
    import jax
    from jax.experimental.shard_map import shard_map
    from jax.sharding import Mesh, NamedSharding, PartitionSpec
    from concourse import bass2jax

    bass2jax.install_neuronx_cc_hook()
    partition_name = (nc.partition_id_tensor.name
                      if nc.partition_id_tensor else None)
    in_names, out_names, out_avals, zero_specs = [], [], [], []
    for alloc in nc.m.functions[0].allocations:
        if not isinstance(alloc, mybir.MemoryLocationSet):
            continue
        name = alloc.memorylocations[0].name
        if alloc.kind == "ExternalInput":
            if name != partition_name:
                in_names.append(name)
        elif alloc.kind == "ExternalOutput":
            shape = tuple(alloc.tensor_shape)
            dtype = mybir.dt.np(alloc.dtype)
            out_names.append(name)
            out_avals.append(jax.core.ShapedArray(shape, dtype))
            zero_specs.append((shape, dtype))
    n_params = len(in_names)
    n_outs = len(out_avals)
    in_names.extend(out_names)
    if partition_name is not None:
        in_names.append(partition_name)
    donate = tuple(range(n_params, n_params + n_outs))

    def _body(*args):
        operands = list(args)
        if partition_name is not None:
            operands.append(bass2jax.partition_id_tensor())
        outs = bass2jax._bass_exec_p.bind(
            *operands,
            out_avals=tuple(out_avals),
            in_names=tuple(in_names),
            out_names=tuple(out_names),
            lowering_input_output_aliases=(),
            sim_require_finite=True,
            sim_require_nnan=True,
            nc=nc,
        )
        return tuple(outs)

    devices = jax.devices()[:NCORES]
    assert len(devices) == NCORES
    mesh = Mesh(np.asarray(devices), ("core",))
    in_specs = (PartitionSpec("core"),) * (n_params + n_outs)
    out_specs = (PartitionSpec("core"),) * n_outs
    sharded = jax.jit(
        shard_map(_body, mesh=mesh, in_specs=in_specs, out_specs=out_specs,
                  check_rep=False),
        donate_argnums=donate, keep_unused=True)
    shard0 = NamedSharding(mesh, PartitionSpec("core"))
    zero_fns = [
        jax.jit(lambda s=shape, d=dtype: jax.numpy.zeros(
            (NCORES * s[0], *s[1:]), d), out_shardings=shard0)
        for shape, dtype in zero_specs
    ]

    def run(in_maps):
        concat_in = [
            np.concatenate(
                [np.asarray(in_maps[c][nm]) for c in range(NCORES)], axis=0)
            for nm in in_names[:n_params]
        ]
        dev_zeros = [fn() for fn in zero_fns]
        out_arrs = sharded(*concat_in, *dev_zeros)
        return [
            {name: np.asarray(out_arrs[i]).reshape(
                NCORES, *out_avals[i].shape)[c]
             for i, name in enumerate(out_names)}
            for c in range(NCORES)
        ]

    return run


def _run_spmd_fast(nc, in_maps):
    global _FAST_EXEC
    if _FAST_EXEC is None:
        _FAST_EXEC = _build_fast_exec(nc)
    return _FAST_EXEC(in_maps)


def kernel(x, Wq, Wk, Wv, Wo, q_norm_w, k_norm_w):
    global _GRAPH, LAST_RESULTS
    x = np.asarray(x, dtype=np.float32)
    Wq = np.asarray(Wq, dtype=np.float32)
    Wk = np.asarray(Wk, dtype=np.float32)
    Wv = np.asarray(Wv, dtype=np.float32)
    Wo = np.asarray(Wo, dtype=np.float32)
    q_norm_w = np.asarray(q_norm_w, dtype=np.float32)
    k_norm_w = np.asarray(k_norm_w, dtype=np.float32)

    rotm_q, rotm_k = _rot_tables(q_norm_w, k_norm_w)
    rot_full = np.concatenate([rotm_q, rotm_k], axis=1)      # [128, 256]
    rotw = rot_full.shape[1] // NCORES

    # int8 quantization: per-input-feature scales, shared across q/k/v,
    # folded into x here (one bf16 rounding instead of two)
    s1 = np.max(np.abs(np.concatenate([Wq, Wk, Wv], 0)), axis=0) / 127.0  # [H]
    xT = np.ascontiguousarray(
        x.reshape(BT, H).T * s1[:, None].astype(np.float32)).astype(BF16)
    invfreq = 1.0 / (THETA ** (np.arange(0, HD, 2, dtype=np.float64) / HD))
    if_col = np.concatenate([invfreq, invfreq]) / TWO_PI     # [128]

    in_maps = []
    for c in range(NCORES):
        kv = c // 2
        w_all = np.concatenate([
            Wq[QH * HD * c:QH * HD * (c + 1)],
            Wk[HD * kv:HD * (kv + 1)],
            Wv[HD * kv:HD * (kv + 1)]], 0)                   # [512, H]
        w8 = np.clip(np.round(w_all / s1[None, :]), -127, 127)
        wqkvT = np.ascontiguousarray(w8.T).astype(np.int8)   # [H, 512]
        wo_cols = Wo[:, QH * HD * c:QH * HD * (c + 1)]       # [H, 256]
        s2 = np.max(np.abs(wo_cols), axis=0) / 127.0         # [256]
        wo8 = np.clip(np.round(wo_cols.T / s2[:, None]), -127, 127)
        woT = np.ascontiguousarray(wo8).astype(np.int8)      # [256, H]
        wc_c = np.empty((128, NWC), dtype=np.int8)
        wc_c[:, OFF_WQKV:OFF_WQKV + 16 * 512] = (
            wqkvT.reshape(16, 128, 512).transpose(1, 0, 2).reshape(128, 16 * 512))
        wc_c[:, OFF_WO:OFF_WO + QH * H] = (
            woT.reshape(QH, 128, H).transpose(1, 0, 2).reshape(128, QH * H))
        wsc_c = np.zeros((128, NSC), dtype=np.float32)
        wsc_c[:, SC_S2:SC_S2 + QH] = s2.reshape(QH, 128).T
        wsc_c[:, SC_IF] = if_col
        wsc_c[:, SC_CWQ] = 1.0 + q_norm_w
        wsc_c[:, SC_CWK] = 1.0 + k_norm_w
        hb = TOKS // 2
        xs_c = np.concatenate(
            [xT[:, b * T + c * hb:b * T + (c + 1) * hb] for b in range(B)],
            axis=1)                                      # [H, 512]
        in_maps.append({
            "xs": np.ascontiguousarray(xs_c),
            "wc": wc_c,
            "wsc": wsc_c,
            "rot": np.ascontiguousarray(rot_full[:, c * rotw:(c + 1) * rotw]),
        })

    if _GRAPH is None:
        _GRAPH = _legalize_waits(_build_graph())

    # honor BASS_TRACE too: profiling-instrumented environments rely on the
    # stock run_bass_kernel_spmd path (NTFF hook) rather than the fast path
    want_trace = bool(int(os.environ.get("ATTN_TRACE", "0"))) or bool(
        os.environ.get("BASS_TRACE"))
    results = None
    if not want_trace:
        try:
            results = _run_spmd_fast(_GRAPH, in_maps)
            LAST_RESULTS = None
        except Exception:
            import traceback
            print("fast exec path failed; falling back:", file=sys.stderr)
            traceback.print_exc(file=sys.stderr)
            results = None
    if results is None:
        try:
            res = run_bass_kernel_spmd(
                _GRAPH, in_maps, core_ids=list(range(NCORES)),
                trace=want_trace)
        except ModuleNotFoundError:
            if not want_trace:
                raise
            # axon NTFF profile hook unavailable in this environment
            res = run_bass_kernel_spmd(
                _GRAPH, in_maps, core_ids=list(range(NCORES)), trace=False)
        LAST_RESULTS = res
        results = res.results
    hb = TOKS // 2
    full = np.concatenate(
        [results[c]["out"][b * hb:(b + 1) * hb]
         for b in range(B) for c in range(NCORES)], axis=0)
    return full.astype(np.float32).reshape(B, T, H)
